# revision 59
# baseline (speedup 1.0000x reference)
"""Bahdanau (additive) attention on Trainium2, 8 NeuronCores.

reference math (per batch b):
    dec_proj = dec @ Wa + Wa_b                      # [H]
    enc_proj = enc[b] @ Ua + Ua_b                   # [S, H]
    energy   = tanh(dec_proj + enc_proj)            # [S, H]
    scores   = energy @ Va + Va_b                   # [S]
    out      = softmax(where(mask == 0, -1e9, scores))

Key optimizations over a straightforward data-parallel split:
  - masked positions produce exactly 0.0 in the reference (exp(-1e9 - max)
    underflows), so the host gathers only the unmasked S positions per batch
    (~50% of them) and the device processes compacted sequences only. The
    softmax itself runs on host in float64 during the scatter.
  - flattened slot layout: the 32 batches are sorted by unmasked length and
    dealt into 4 "slots" of 8 (one batch per core per slot). Each core's
    device sequence is the concatenation of its 4 slots, so slot boundaries
    (and hence the per-slot tanh-bias activation splits) are compile-time
    constants shared by all 8 SPMD cores, while the per-slot padding is the
    max *within a rank-group of 8* instead of the global max.
  - mixed-precision matmul: h-planes 0-1 (256 of 1024 contraction dims) run
    as one fp8e4m3 DoubleRow matmul pair at 2x PE rate; planes 2-7 stay
    bf16. Measured end-to-end rel err 1.7e-2 < 2e-2 (vs 2.7e-3 all-bf16).
    Ua is pre-scaled x32 (both fp8 and bf16 parts) so fp8 operands sit in
    e4m3's normal range; the tanh activation applies scale=1/32. Only the
    first matmul into a PSUM bank carries start=True — the zero-region is
    bank-granular, a second start wipes the sibling DR's columns.
  - chunk-outer / kt-mid / plane-inner emission: each 512-col chunk runs all
    8 output k-blocks before moving on. Startup needs only chunk 0's data,
    the PSUM working set is one bank per k-block, and per-chunk scores
    stream out across the whole kernel. Chunk 0 goes DR-first (12 fp8
    matmuls off a 0.3 MB footprint) so the PE has real work while the bf16
    weights are still in flight.
  - startup DMAs stay fine-grained (one descriptor per enc plane-slice) so
    transfers spread across all 16 DMA engines in parallel — consolidated
    multi-plane descriptors measured slower. Two rings (SP + GpSimd queues,
    keeping ScalarE free for tanh), ordered by first use: fp8 weights + enc
    chunk 0, kt0-1 bf16 weights + biases, chunk 1-2 slices, bulk tail.
  - per-partition tanh bias cbias = dec@Wa + Wa_b + Ua_b precomputed on host
    (0.05% of flops); DVE folds the Va contraction per chunk; PE finishes
    with a ones-vector partition-sum per chunk, emitted one chunk late so it
    never stalls the matmul stream. The tail chunk folds Va by PE matmul
    (va as stationary, accumulated over kt in one PSUM sliver) instead,
    removing the serial tanh->DVE->sum drain chain from the critical tail.
"""

import numpy as np
import ml_dtypes

B, S, H = 32, 2048, 1024
NCORES = 8
NSLOT = 4  # batches per core
P = 128
CW = 512  # matmul chunk width == one fp32 PSUM bank
KT = H // P
HT = H // P
NF8 = 2  # h-planes 0..NF8-1 go through the fp8 DoubleRow path
NB16 = HT - NF8
WSCALE = 32.0  # host pre-scale on Ua (both parts); tanh applies 1/WSCALE


def build_kernel(nc, T, bounds):
    """T: per-core device sequence length (multiple of 16).
    bounds: slot start offsets + T, len NSLOT+1, compile-time constants."""
    from contextlib import ExitStack
    import concourse.tile as tile
    from concourse import mybir

    f32, bf16 = mybir.dt.float32, mybir.dt.bfloat16
    f32r = mybir.dt.float32r
    f8 = mybir.dt.float8e4
    DR = mybir.MatmulPerfMode.DoubleRow
    Tanh = mybir.ActivationFunctionType.Tanh
    Copy = mybir.ActivationFunctionType.Copy

    chunks = [CW] * (T // CW) + ([T % CW] if T % CW else [])
    NCH = len(chunks)
    coff = [sum(chunks[:i]) for i in range(NCH + 1)]

    def act_ranges(c):
        """(col0, col1, slot) pieces of chunk c split at slot boundaries."""
        c0, c1 = coff[c], coff[c] + chunks[c]
        out = []
        for j in range(NSLOT):
            lo, hi = max(c0, bounds[j]), min(c1, bounds[j + 1])
            if lo < hi:
                out.append((lo, hi, j))
        return out

    enc8 = nc.dram_tensor("enc8", [P, NF8, T], f8, kind="ExternalInput").ap()
    encb = nc.dram_tensor("encb", [P, NB16, T], bf16, kind="ExternalInput").ap()
    ua8 = nc.dram_tensor("ua8", [P, KT, NF8, P], f8, kind="ExternalInput").ap()
    uab = nc.dram_tensor("uab", [P, KT, NB16 * P], bf16, kind="ExternalInput").ap()
    # cbf packs cbias [P, KT*NSLOT] ++ va [P, KT], both f32
    cbf = nc.dram_tensor("cbf", [P, KT * NSLOT + KT], f32, kind="ExternalInput").ap()
    vab = nc.dram_tensor("vab", [P, KT], bf16, kind="ExternalInput").ap()
    out = nc.dram_tensor("scores", [1, T], f32, kind="ExternalOutput").ap()

    with ExitStack() as ctx:
        tc = ctx.enter_context(tile.TileContext(nc))
        const = ctx.enter_context(tc.tile_pool(name="const", bufs=1))
        enp = ctx.enter_context(tc.tile_pool(name="energy", bufs=4))
        mmp = ctx.enter_context(tc.tile_pool(name="mm", bufs=6, space="PSUM"))
        scp = ctx.enter_context(tc.tile_pool(name="sc", bufs=2, space="PSUM"))

        A, Bng = nc.sync, nc.gpsimd  # the two HWDGE rings

        # ---- PE clock warm-up ----
        # bf16 junk via two GpSimd memsets only (DVE's preamble is slow), so
        # the first junk matmul issues ~1.5us in; ones_sb (f32r, needed by the
        # first psum-sum at ~15us) goes through DVE's f32->f32r copy dance.
        JW = 256
        junk_rhs = const.tile([P, JW], bf16, tag="junk")
        nc.gpsimd.memset(junk_rhs[:], 0.0)
        junk_ones = const.tile([P, 1], bf16, tag="jones")
        nc.gpsimd.memset(junk_ones[:], 1.0)
        ones_f = const.tile([P, 1], f32, tag="onesf")
        nc.vector.memset(ones_f[:], 1.0)
        ones_sb = const.tile([P, 1], f32r, tag="ones")
        nc.vector.tensor_copy(ones_sb[:], ones_f[:])
        for j in range(14):
            jt = scp.tile([1, CW], f32, tag="sc", name=f"junk_{j}")
            nc.tensor.matmul(jt[:, 0:JW], junk_ones[:], junk_rhs[:], start=True, stop=True)

        # ---- SBUF tiles ----
        enc8_t = const.tile([P, NF8, T], f8, tag="enc8")
        encb_t = const.tile([P, NB16, T], bf16, tag="encb")
        ua8_t = const.tile([P, KT, NF8, P], f8, tag="ua8")
        uab_t = const.tile([P, KT, NB16 * P], bf16, tag="uab")
        cbf_t = const.tile([P, KT * NSLOT + KT], f32, tag="cbf")
        vab_t = const.tile([P, KT], bf16, tag="vab")
        acc = const.tile([P, T], f32r, tag="acc")
        prow = const.tile([1, T], f32, tag="prow")

        def cbias_col(kt, sl):
            i = kt * NSLOT + sl
            return cbf_t[:, i : i + 1]

        def va_col(kt):
            i = KT * NSLOT + kt
            return cbf_t[:, i : i + 1]

        # ---- startup DMAs: fine-grained (parallel engine delivery), ordered
        # by first use: ua8+enc8-c0 feed the chunk-0 DR-first block, uab0-1
        # right behind for its bf16 sweeps, then chunk 1-2 slices, bulk tail.
        A.dma_start(ua8_t[:], ua8)
        Bng.dma_start(enc8_t[:, :, 0:CW], enc8[:, :, 0:CW])
        A.dma_start(uab_t[:, 0, :], uab[:, 0, :])
        Bng.dma_start(cbf_t[:], cbf)
        A.dma_start(uab_t[:, 1, :], uab[:, 1, :])
        for j in range(NB16):
            r = (Bng, A)[j % 2]
            r.dma_start(encb_t[:, j, 0:CW], encb[:, j, 0:CW])
            if j < 3:
                (A, Bng)[j % 2].dma_start(uab_t[:, j + 2, :], uab[:, j + 2, :])
        Bng.dma_start(enc8_t[:, :, CW : 2 * CW], enc8[:, :, CW : 2 * CW])
        for j in range(NB16):
            r = (A, Bng)[j % 2]
            r.dma_start(encb_t[:, j, CW : 2 * CW], encb[:, j, CW : 2 * CW])
            if j < 3:
                (Bng, A)[j % 2].dma_start(uab_t[:, j + 5, :], uab[:, j + 5, :])
        A.dma_start(enc8_t[:, :, 2 * CW : 3 * CW], enc8[:, :, 2 * CW : 3 * CW])
        Bng.dma_start(vab_t[:], vab)
        for j in range(NB16):
            (Bng, A)[j % 2].dma_start(encb_t[:, j, 2 * CW : 3 * CW],
                                      encb[:, j, 2 * CW : 3 * CW])
        t0 = 3 * CW
        if t0 < T:
            tm = (t0 + T) // 2 // 16 * 16
            A.dma_start(enc8_t[:, :, t0:tm], enc8[:, :, t0:tm])
            Bng.dma_start(enc8_t[:, :, tm:T], enc8[:, :, tm:T])
            for j in range(NB16):
                r, rr = ((A, Bng), (Bng, A))[j % 2]
                r.dma_start(encb_t[:, j, t0:tm], encb[:, j, t0:tm])
                rr.dma_start(encb_t[:, j, tm:T], encb[:, j, tm:T])

        # ---- helpers ----
        def dr_pair(mm, kt, c0, w, first):
            if w <= 256:
                # rhs free = 2w <= 512: one DR matmul covers the whole chunk
                nc.tensor.matmul(mm[:, 0:w], ua8_t[:, kt, :, :],
                                 enc8_t[:, :, c0 : c0 + w],
                                 start=first, stop=False, perf_mode=DR)
                return
            h = w // 2
            nc.tensor.matmul(mm[:, 0:h], ua8_t[:, kt, :, :],
                             enc8_t[:, :, c0 : c0 + h],
                             start=first, stop=False, perf_mode=DR)
            nc.tensor.matmul(mm[:, h:w], ua8_t[:, kt, :, :],
                             enc8_t[:, :, c0 + h : c0 + w],
                             start=False, stop=False, perf_mode=DR)

        def bf_sweep(mm, kt, c0, w):
            for j in range(NB16):
                nc.tensor.matmul(mm[:, 0:w],
                                 uab_t[:, kt, j * P : (j + 1) * P],
                                 encb_t[:, j, c0 : c0 + w],
                                 start=False, stop=(j == NB16 - 1))

        def tanh_fold(mm, en, kt, c, split=False):
            c0, w = coff[c], chunks[c]
            # split=True halves the tanh/fold so the drain chain pipelines:
            # used for the last k-block of the last regular chunk, whose
            # tanh->fold->psum-sum chain gates the kernel tail
            pieces = []
            for (r0, r1, sl) in act_ranges(c):
                if split and r1 - r0 > 256:
                    m = (r0 + r1) // 2
                    pieces += [(r0, m, sl), (m, r1, sl)]
                else:
                    pieces.append((r0, r1, sl))
            for i, (r0, r1, sl) in enumerate(pieces):
                nc.scalar.activation(en[:, r0 - c0 : r1 - c0],
                                     mm[:, r0 - c0 : r1 - c0], Tanh,
                                     bias=cbias_col(kt, sl), scale=1.0 / WSCALE)
                if not split:
                    continue
                if kt == 0:
                    nc.vector.tensor_scalar(acc[:, r0:r1], en[:, r0 - c0 : r1 - c0],
                                            va_col(0), None,
                                            op0=mybir.AluOpType.mult)
                else:
                    nc.vector.scalar_tensor_tensor(acc[:, r0:r1],
                                                   en[:, r0 - c0 : r1 - c0],
                                                   va_col(kt), acc[:, r0:r1],
                                                   op0=mybir.AluOpType.mult,
                                                   op1=mybir.AluOpType.add)
            if split:
                return
            if kt == 0:
                nc.vector.tensor_scalar(acc[:, c0 : c0 + w], en[:, 0:w],
                                        va_col(0), None,
                                        op0=mybir.AluOpType.mult)
            else:
                nc.vector.scalar_tensor_tensor(acc[:, c0 : c0 + w], en[:, 0:w],
                                               va_col(kt), acc[:, c0 : c0 + w],
                                               op0=mybir.AluOpType.mult,
                                               op1=mybir.AluOpType.add)

        pend_sum = []  # chunk idx awaiting its psum-sum matmul
        pend_ship = []  # (chunk idx, sc tile) awaiting the prow copy
        shipped = [0]  # prow columns already sent to DRAM

        def psum_sum(c):
            c0, w = coff[c], chunks[c]
            sc = scp.tile([1, CW], f32, tag="sc")
            nc.tensor.matmul(sc[:, 0:w], ones_sb[:], acc[:, c0 : c0 + w],
                             start=True, stop=True)
            return sc

        def ship(c, sc):
            c0, w = coff[c], chunks[c]
            if c >= NCH - 3:
                # near the tail, keep ScalarE free for the last tanh burst
                nc.vector.tensor_copy(prow[0:1, c0 : c0 + w], sc[:, 0:w])
            else:
                nc.scalar.activation(prow[0:1, c0 : c0 + w], sc[:, 0:w], Copy)
            if c in (NCH - 5, NCH - 3, NCH - 2):
                # progressive output: the final transfer stays tiny
                e = coff[c + 1]
                A.dma_start(out[0:1, shipped[0] : e], prow[0:1, shipped[0] : e])
                shipped[0] = e

        # ---- chunk 0: DR-first so the PE has work off a tiny DMA footprint ----
        c0w = chunks[0]
        mm0 = [mmp.tile([P, CW], f32, tag="mm", name=f"mm{kt}_0") for kt in range(KT)]
        en0 = {}
        for kt in range(6):
            dr_pair(mm0[kt], kt, 0, c0w, first=True)
        for kt in range(KT):
            if kt >= 6:
                dr_pair(mm0[kt], kt, 0, c0w, first=True)
            bf_sweep(mm0[kt], kt, 0, c0w)
            en0[kt] = enp.tile([P, CW], bf16, tag="en", name=f"en{kt}_0")
            tanh_fold(mm0[kt], en0[kt], kt, 0)
        pend_sum.append(0)

        # ---- chunks 1..NCH-2: uniform kt-groups ----
        for c in range(1, NCH - 1):
            c0, w = coff[c], chunks[c]
            for kt in range(KT):
                mm = mmp.tile([P, CW], f32, tag="mm", name=f"mm{kt}_{c}")
                dr_pair(mm, kt, c0, w, first=True)
                bf_sweep(mm, kt, c0, w)
                if kt == 2 and pend_sum:
                    cc = pend_sum.pop(0)
                    pend_ship.append((cc, psum_sum(cc)))
                if kt == 4 and pend_ship:
                    cc, sc = pend_ship.pop(0)
                    ship(cc, sc)
                en = enp.tile([P, CW], bf16, tag="en", name=f"en{kt}_{c}")
                tanh_fold(mm, en, kt, c)
            pend_sum.append(c)

        # ---- tail chunk: Va-fold on PE (no serial DVE chain). The fold
        # matmuls (va.T @ en_kt accumulated into one PSUM sliver) ride the
        # tail matmul stream with a 2-group lag so only kt6/kt7's folds can
        # ever wait on a tanh at the very end. ----
        ct = NCH - 1
        c0, w = coff[ct], chunks[ct]
        en_t = {}
        sct = scp.tile([1, CW], f32, tag="sc")

        def fold_mm(kt):
            nc.tensor.matmul(sct[:, 0:w], vab_t[:, kt : kt + 1], en_t[kt][:, 0:w],
                             start=(kt == 0), stop=(kt == KT - 1))

        for kt in range(KT):
            mm = mmp.tile([P, CW], f32, tag="mm", name=f"mm{kt}_{ct}")
            dr_pair(mm, kt, c0, w, first=True)
            bf_sweep(mm, kt, c0, w)
            if kt == 2 and pend_sum:
                cc = pend_sum.pop(0)
                pend_ship.append((cc, psum_sum(cc)))
            if kt == 4 and pend_ship:
                cc, sc = pend_ship.pop(0)
                ship(cc, sc)
            # dedicated tiles: an enp-pool tile here would chain the tail
            # tanhs behind the fold-matmuls' consumption of earlier slots
            en = const.tile([P, CW], bf16, tag=f"ent{kt}", name=f"en{kt}_t")
            en_t[kt] = en
            for (r0, r1, sl) in act_ranges(ct):
                nc.scalar.activation(en[:, r0 - c0 : r1 - c0],
                                     mm[:, r0 - c0 : r1 - c0], Tanh,
                                     bias=cbias_col(kt, sl), scale=1.0 / WSCALE)
            if kt >= 2:
                fold_mm(kt - 2)
        # drain any leftover mid-chunk bookkeeping
        for cc, sc in pend_ship:
            ship(cc, sc)
        pend_ship = []
        for cc in pend_sum:
            if cc != ct:
                sc = psum_sum(cc)
                ship(cc, sc)
        pend_sum = []
        fold_mm(KT - 2)
        fold_mm(KT - 1)
        nc.vector.tensor_copy(prow[0:1, c0 : c0 + w], sct[:, 0:w])
        A.dma_start(out[0:1, shipped[0] : T], prow[0:1, shipped[0] : T])
        # ---- tail clock-keeper ----
        # The DVFS governor cuts the clock to 50% ~1.5us after PE activity
        # stops (ham type-1 record at t_end+1.5us), slowing the final output
        # DMA round-trip and the end-of-program barriers. Junk matmuls fill
        # the otherwise-idle DMA-wait window to hold full clock through the
        # epilogue; they retire well before the DMA quiesce completes.
        for j in range(16):
            jt = scp.tile([1, CW], f32, tag="sc", name=f"tailjunk_{j}")
            nc.tensor.matmul(jt[:, 0:JW], junk_ones[:], junk_rhs[:],
                             start=True, stop=True)

    return nc


def make_nc(T, bounds):
    from concourse import bacc

    nc = bacc.Bacc("TRN2", target_bir_lowering=False)
    build_kernel(nc, T, bounds)
    nc.compile()
    return nc


def host_prep(decoder_hidden, encoder_outputs, mask, Wa_w, Wa_b, Ua_w, Ua_b, Va_w,
              n_cores=NCORES):
    """Compact, slot-balance, quantize and lay out inputs for the device.

    Returns (in_maps, T, bounds, placement) where placement[core] is a list
    of (batch, n_kept, col_offset) per slot for the host-side scatter."""
    bf = ml_dtypes.bfloat16
    e4 = ml_dtypes.float8_e4m3fn
    b_total, s, h = encoder_outputs.shape

    mask_np = np.asarray(mask)
    idxs = [np.nonzero(mask_np[b])[0] for b in range(b_total)]
    s_eff = np.array([len(i) for i in idxs])

    # sort batches by length desc; slot j takes ranks [8j, 8j+8), one per core
    order = np.argsort(-s_eff, kind="stable")
    L = []
    assign = []  # assign[j][c] = batch id
    for j in range(NSLOT):
        grp = order[n_cores * j : n_cores * (j + 1)]
        L.append(int(min(-(-max(s_eff[grp].max(), 1) // 16) * 16, s)))
        assign.append(list(grp))
    T = sum(L)
    bounds = [0]
    for lj in L:
        bounds.append(bounds[-1] + lj)

    f32 = np.float32
    dec = np.asarray(decoder_hidden, f32)
    enc = np.asarray(encoder_outputs, f32)
    Ua = np.asarray(Ua_w, f32)
    cb_full = (dec @ np.asarray(Wa_w, f32) + np.asarray(Wa_b, f32)
               + np.asarray(Ua_b, f32))  # [B, H]

    # weights, replicated
    ua_s = WSCALE * Ua
    ua8 = np.ascontiguousarray(
        ua_s[0 : NF8 * P, :].reshape(NF8, P, KT, P).transpose(1, 2, 0, 3)
    ).astype(e4)  # [P, KT, NF8, P]
    uab = np.ascontiguousarray(
        ua_s[NF8 * P :, :].reshape(NB16, P, KT, P).transpose(1, 2, 0, 3)
        .reshape(P, KT, NB16 * P)
    ).astype(bf)
    va_sb = np.ascontiguousarray(np.asarray(Va_w, f32).reshape(KT, P).T)  # [P, KT]
    vab = va_sb.astype(bf)

    in_maps = []
    placement = []
    for c in range(n_cores):
        enc8 = np.zeros((P, NF8, T), e4)
        encb = np.zeros((P, NB16, T), bf)
        cbias = np.zeros((P, KT * NSLOT), f32)
        place = []
        for j in range(NSLOT):
            b = assign[j][c]
            n = min(int(s_eff[b]), L[j])
            o = bounds[j]
            et = enc[b][idxs[b][:n]].T  # [H, n]
            enc8[:, :, o : o + n] = et[0 : NF8 * P].reshape(NF8, P, n).transpose(1, 0, 2).astype(e4)
            encb[:, :, o : o + n] = et[NF8 * P :].reshape(NB16, P, n).transpose(1, 0, 2).astype(bf)
            cbias[:, j::NSLOT] = cb_full[b].reshape(KT, P).T  # col kt*NSLOT+j
            place.append((int(b), n, o))
        cbf = np.concatenate([cbias, va_sb], axis=1)
        in_maps.append(dict(enc8=enc8, encb=encb, ua8=ua8, uab=uab,
                            cbf=cbf, vab=vab))
        placement.append(place)
    return in_maps, T, bounds, placement


def scatter_output(core_outs, placement, idxs_all, b_total, s_full):
    """Softmax the per-core score rows (host, float64) and scatter back to
    the full [B, S] output. Masked positions are exactly 0.0, matching the
    reference's underflowed exp."""
    out = np.zeros((b_total, s_full), np.float32)
    for c, row in enumerate(core_outs):
        for (b, n, o) in placement[c]:
            if n == 0:
                continue
            r = row[o : o + n].astype(np.float64)
            e = np.exp(r - r.max())
            out[b, idxs_all[b][:n]] = (e / e.sum()).astype(np.float32)
    return out


_NC_CACHE = {}


def run(inputs, trace=False, **spmd_kwargs):
    """Run on the 8 NeuronCores; returns (full_output, BassKernelResults)."""
    from concourse.bass_utils import run_bass_kernel_spmd

    mask_np = np.asarray(inputs["mask"])
    idxs_all = [np.nonzero(mask_np[b])[0] for b in range(mask_np.shape[0])]
    in_maps, T, bounds, placement = host_prep(
        inputs["decoder_hidden"],
        inputs["encoder_outputs"],
        inputs["mask"],
        inputs["Wa_w"],
        inputs["Wa_b"],
        inputs["Ua_w"],
        inputs["Ua_b"],
        inputs["Va_w"],
    )
    key = (T, tuple(bounds))
    if key not in _NC_CACHE:
        _NC_CACHE[key] = make_nc(T, bounds)
    nc = _NC_CACHE[key]
    res = run_bass_kernel_spmd(
        nc, in_maps, list(range(NCORES)), trace=trace, **spmd_kwargs
    )
    outs = [np.asarray(r["scores"], np.float32).reshape(-1) for r in res.results]
    return scatter_output(outs, placement, idxs_all, B, S), res


def kernel(**inputs) -> np.ndarray:
    out, _ = run(inputs, trace=False)
    return out


# revision 60
# speedup vs baseline: 1.0240x; 1.0240x over previous
"""Bahdanau (additive) attention on Trainium2, 8 NeuronCores.

reference math (per batch b):
    dec_proj = dec @ Wa + Wa_b                      # [H]
    enc_proj = enc[b] @ Ua + Ua_b                   # [S, H]
    energy   = tanh(dec_proj + enc_proj)            # [S, H]
    scores   = energy @ Va + Va_b                   # [S]
    out      = softmax(where(mask == 0, -1e9, scores))

Key optimizations over a straightforward data-parallel split:
  - masked positions produce exactly 0.0 in the reference (exp(-1e9 - max)
    underflows), so the host gathers only the unmasked S positions per batch
    (~50% of them) and the device processes compacted sequences only. The
    softmax itself runs on host in float64 during the scatter.
  - flattened slot layout: the 32 batches are sorted by unmasked length and
    dealt into 4 "slots" of 8 (one batch per core per slot). Each core's
    device sequence is the concatenation of its 4 slots, so slot boundaries
    (and hence the per-slot tanh-bias activation splits) are compile-time
    constants shared by all 8 SPMD cores, while the per-slot padding is the
    max *within a rank-group of 8* instead of the global max.
  - mixed-precision matmul: h-planes 0-1 (256 of 1024 contraction dims) run
    as one fp8e4m3 DoubleRow matmul pair at 2x PE rate; planes 2-7 stay
    bf16. Measured end-to-end rel err 1.7e-2 < 2e-2 (vs 2.7e-3 all-bf16).
    Ua is pre-scaled x32 (both fp8 and bf16 parts) so fp8 operands sit in
    e4m3's normal range; the tanh activation applies scale=1/32. Only the
    first matmul into a PSUM bank carries start=True — the zero-region is
    bank-granular, a second start wipes the sibling DR's columns.
  - chunk-outer / kt-mid / plane-inner emission: each 512-col chunk runs all
    8 output k-blocks before moving on. Startup needs only chunk 0's data,
    the PSUM working set is one bank per k-block, and per-chunk scores
    stream out across the whole kernel. Chunk 0 goes DR-first (12 fp8
    matmuls off a 0.3 MB footprint) so the PE has real work while the bf16
    weights are still in flight.
  - startup DMAs stay fine-grained (one descriptor per enc plane-slice) so
    transfers spread across all 16 DMA engines in parallel — consolidated
    multi-plane descriptors measured slower. Two rings (SP + GpSimd queues,
    keeping ScalarE free for tanh), ordered by first use: fp8 weights + enc
    chunk 0, kt0-1 bf16 weights + biases, chunk 1-2 slices, bulk tail.
  - per-partition tanh bias cbias = dec@Wa + Wa_b + Ua_b precomputed on host
    (0.05% of flops); DVE folds the Va contraction per chunk; PE finishes
    with a ones-vector partition-sum per chunk, emitted one chunk late so it
    never stalls the matmul stream. The tail chunk folds Va by PE matmul
    (va as stationary, accumulated over kt in one PSUM sliver) instead,
    removing the serial tanh->DVE->sum drain chain from the critical tail.
"""

import numpy as np
import ml_dtypes

B, S, H = 32, 2048, 1024
NCORES = 8
NSLOT = 4  # batches per core
P = 128
CW = 512  # matmul chunk width == one fp32 PSUM bank
KT = H // P
HT = H // P
NF8 = 2  # h-planes 0..NF8-1 go through the fp8 DoubleRow path
NB16 = HT - NF8
WSCALE = 32.0  # host pre-scale on Ua (both parts); tanh applies 1/WSCALE


def build_kernel(nc, T, bounds):
    """T: per-core device sequence length (multiple of 16).
    bounds: slot start offsets + T, len NSLOT+1, compile-time constants."""
    from contextlib import ExitStack
    import concourse.tile as tile
    from concourse import mybir

    f32, bf16 = mybir.dt.float32, mybir.dt.bfloat16
    f32r = mybir.dt.float32r
    f8 = mybir.dt.float8e4
    DR = mybir.MatmulPerfMode.DoubleRow
    Tanh = mybir.ActivationFunctionType.Tanh
    Copy = mybir.ActivationFunctionType.Copy

    chunks = [CW] * (T // CW) + ([T % CW] if T % CW else [])
    NCH = len(chunks)
    coff = [sum(chunks[:i]) for i in range(NCH + 1)]

    def act_ranges(c):
        """(col0, col1, slot) pieces of chunk c split at slot boundaries."""
        c0, c1 = coff[c], coff[c] + chunks[c]
        out = []
        for j in range(NSLOT):
            lo, hi = max(c0, bounds[j]), min(c1, bounds[j + 1])
            if lo < hi:
                out.append((lo, hi, j))
        return out

    enc8 = nc.dram_tensor("enc8", [P, NF8, T], f8, kind="ExternalInput").ap()
    encb = nc.dram_tensor("encb", [P, NB16, T], bf16, kind="ExternalInput").ap()
    ua8 = nc.dram_tensor("ua8", [P, KT, NF8, P], f8, kind="ExternalInput").ap()
    uab = nc.dram_tensor("uab", [P, KT, NB16 * P], bf16, kind="ExternalInput").ap()
    # cbf packs cbias [P, KT*NSLOT] ++ va [P, KT], both f32
    cbf = nc.dram_tensor("cbf", [P, KT * NSLOT + KT], f32, kind="ExternalInput").ap()
    vab = nc.dram_tensor("vab", [P, KT], bf16, kind="ExternalInput").ap()
    out = nc.dram_tensor("scores", [1, T], f32, kind="ExternalOutput").ap()

    with ExitStack() as ctx:
        tc = ctx.enter_context(tile.TileContext(nc))
        const = ctx.enter_context(tc.tile_pool(name="const", bufs=1))
        enp = ctx.enter_context(tc.tile_pool(name="energy", bufs=4))
        mmp = ctx.enter_context(tc.tile_pool(name="mm", bufs=6, space="PSUM"))
        scp = ctx.enter_context(tc.tile_pool(name="sc", bufs=2, space="PSUM"))

        A, Bng = nc.sync, nc.gpsimd  # the two HWDGE rings

        # ---- PE clock warm-up ----
        # bf16 junk via two GpSimd memsets only (DVE's preamble is slow), so
        # the first junk matmul issues ~1.5us in; ones_sb (f32r, needed by the
        # first psum-sum at ~15us) goes through DVE's f32->f32r copy dance.
        JW = 256
        junk_rhs = const.tile([P, JW], bf16, tag="junk")
        nc.gpsimd.memset(junk_rhs[:], 0.0)
        junk_ones = const.tile([P, 1], bf16, tag="jones")
        nc.gpsimd.memset(junk_ones[:], 1.0)
        ones_f = const.tile([P, 1], f32, tag="onesf")
        nc.vector.memset(ones_f[:], 1.0)
        ones_sb = const.tile([P, 1], f32r, tag="ones")
        nc.vector.tensor_copy(ones_sb[:], ones_f[:])
        for j in range(14):
            jt = scp.tile([1, CW], f32, tag="sc", name=f"junk_{j}")
            nc.tensor.matmul(jt[:, 0:JW], junk_ones[:], junk_rhs[:], start=True, stop=True)

        # ---- SBUF tiles ----
        enc8_t = const.tile([P, NF8, T], f8, tag="enc8")
        encb_t = const.tile([P, NB16, T], bf16, tag="encb")
        ua8_t = const.tile([P, KT, NF8, P], f8, tag="ua8")
        uab_t = const.tile([P, KT, NB16 * P], bf16, tag="uab")
        cbf_t = const.tile([P, KT * NSLOT + KT], f32, tag="cbf")
        vab_t = const.tile([P, KT], bf16, tag="vab")
        acc = const.tile([P, T], f32r, tag="acc")
        prow = const.tile([1, T], f32, tag="prow")

        def cbias_col(kt, sl):
            i = kt * NSLOT + sl
            return cbf_t[:, i : i + 1]

        def va_col(kt):
            i = KT * NSLOT + kt
            return cbf_t[:, i : i + 1]

        # ---- startup DMAs: fine-grained (parallel engine delivery), ordered
        # by first use: ua8+enc8-c0 feed the chunk-0 DR-first block, uab0-1
        # right behind for its bf16 sweeps, then chunk 1-2 slices, bulk tail.
        A.dma_start(ua8_t[:], ua8)
        Bng.dma_start(enc8_t[:, :, 0:CW], enc8[:, :, 0:CW])
        A.dma_start(uab_t[:, 0, :], uab[:, 0, :])
        Bng.dma_start(cbf_t[:], cbf)
        A.dma_start(uab_t[:, 1, :], uab[:, 1, :])
        for j in range(NB16):
            r = (Bng, A)[j % 2]
            r.dma_start(encb_t[:, j, 0:CW], encb[:, j, 0:CW])
            if j < 3:
                (A, Bng)[j % 2].dma_start(uab_t[:, j + 2, :], uab[:, j + 2, :])
        Bng.dma_start(enc8_t[:, :, CW : 2 * CW], enc8[:, :, CW : 2 * CW])
        for j in range(NB16):
            r = (A, Bng)[j % 2]
            r.dma_start(encb_t[:, j, CW : 2 * CW], encb[:, j, CW : 2 * CW])
            if j < 3:
                (Bng, A)[j % 2].dma_start(uab_t[:, j + 5, :], uab[:, j + 5, :])
        A.dma_start(enc8_t[:, :, 2 * CW : 3 * CW], enc8[:, :, 2 * CW : 3 * CW])
        Bng.dma_start(vab_t[:], vab)
        for j in range(NB16):
            (Bng, A)[j % 2].dma_start(encb_t[:, j, 2 * CW : 3 * CW],
                                      encb[:, j, 2 * CW : 3 * CW])
        t0 = 3 * CW
        if t0 < T:
            tm = (t0 + T) // 2 // 16 * 16
            A.dma_start(enc8_t[:, :, t0:tm], enc8[:, :, t0:tm])
            Bng.dma_start(enc8_t[:, :, tm:T], enc8[:, :, tm:T])
            for j in range(NB16):
                r, rr = ((A, Bng), (Bng, A))[j % 2]
                r.dma_start(encb_t[:, j, t0:tm], encb[:, j, t0:tm])
                rr.dma_start(encb_t[:, j, tm:T], encb[:, j, tm:T])

        # ---- helpers ----
        def dr_pair(mm, kt, c0, w, first):
            if w <= 256:
                # rhs free = 2w <= 512: one DR matmul covers the whole chunk
                nc.tensor.matmul(mm[:, 0:w], ua8_t[:, kt, :, :],
                                 enc8_t[:, :, c0 : c0 + w],
                                 start=first, stop=False, perf_mode=DR)
                return
            h = w // 2
            nc.tensor.matmul(mm[:, 0:h], ua8_t[:, kt, :, :],
                             enc8_t[:, :, c0 : c0 + h],
                             start=first, stop=False, perf_mode=DR)
            nc.tensor.matmul(mm[:, h:w], ua8_t[:, kt, :, :],
                             enc8_t[:, :, c0 + h : c0 + w],
                             start=False, stop=False, perf_mode=DR)

        def bf_sweep(mm, kt, c0, w):
            for j in range(NB16):
                nc.tensor.matmul(mm[:, 0:w],
                                 uab_t[:, kt, j * P : (j + 1) * P],
                                 encb_t[:, j, c0 : c0 + w],
                                 start=False, stop=(j == NB16 - 1))

        def tanh_fold(mm, en, kt, c, split=False):
            c0, w = coff[c], chunks[c]
            # split=True halves the tanh/fold so the drain chain pipelines:
            # used for the last k-block of the last regular chunk, whose
            # tanh->fold->psum-sum chain gates the kernel tail
            pieces = []
            for (r0, r1, sl) in act_ranges(c):
                if split and r1 - r0 > 256:
                    m = (r0 + r1) // 2
                    pieces += [(r0, m, sl), (m, r1, sl)]
                else:
                    pieces.append((r0, r1, sl))
            for i, (r0, r1, sl) in enumerate(pieces):
                nc.scalar.activation(en[:, r0 - c0 : r1 - c0],
                                     mm[:, r0 - c0 : r1 - c0], Tanh,
                                     bias=cbias_col(kt, sl), scale=1.0 / WSCALE)
                if not split:
                    continue
                if kt == 0:
                    nc.vector.tensor_scalar(acc[:, r0:r1], en[:, r0 - c0 : r1 - c0],
                                            va_col(0), None,
                                            op0=mybir.AluOpType.mult)
                else:
                    nc.vector.scalar_tensor_tensor(acc[:, r0:r1],
                                                   en[:, r0 - c0 : r1 - c0],
                                                   va_col(kt), acc[:, r0:r1],
                                                   op0=mybir.AluOpType.mult,
                                                   op1=mybir.AluOpType.add)
            if split:
                return
            if kt == 0:
                nc.vector.tensor_scalar(acc[:, c0 : c0 + w], en[:, 0:w],
                                        va_col(0), None,
                                        op0=mybir.AluOpType.mult)
            else:
                nc.vector.scalar_tensor_tensor(acc[:, c0 : c0 + w], en[:, 0:w],
                                               va_col(kt), acc[:, c0 : c0 + w],
                                               op0=mybir.AluOpType.mult,
                                               op1=mybir.AluOpType.add)

        pend_sum = []  # chunk idx awaiting its psum-sum matmul
        pend_ship = []  # (chunk idx, sc tile) awaiting the prow copy
        shipped = [0]  # prow columns already sent to DRAM

        def psum_sum(c):
            c0, w = coff[c], chunks[c]
            sc = scp.tile([1, CW], f32, tag="sc")
            nc.tensor.matmul(sc[:, 0:w], ones_sb[:], acc[:, c0 : c0 + w],
                             start=True, stop=True)
            return sc

        def ship(c, sc):
            c0, w = coff[c], chunks[c]
            if c >= NCH - 3:
                # near the tail, keep ScalarE free for the last tanh burst
                nc.vector.tensor_copy(prow[0:1, c0 : c0 + w], sc[:, 0:w])
            else:
                nc.scalar.activation(prow[0:1, c0 : c0 + w], sc[:, 0:w], Copy)
            if c in (NCH - 5, NCH - 3, NCH - 2):
                # progressive output: the final transfer stays tiny
                e = coff[c + 1]
                A.dma_start(out[0:1, shipped[0] : e], prow[0:1, shipped[0] : e])
                shipped[0] = e

        # ---- chunk 0: DR-first so the PE has work off a tiny DMA footprint ----
        c0w = chunks[0]
        mm0 = [mmp.tile([P, CW], f32, tag="mm", name=f"mm{kt}_0") for kt in range(KT)]
        en0 = {}
        for kt in range(6):
            dr_pair(mm0[kt], kt, 0, c0w, first=True)
        for kt in range(KT):
            if kt >= 6:
                dr_pair(mm0[kt], kt, 0, c0w, first=True)
            bf_sweep(mm0[kt], kt, 0, c0w)
            en0[kt] = enp.tile([P, CW], bf16, tag="en", name=f"en{kt}_0")
            tanh_fold(mm0[kt], en0[kt], kt, 0)
        pend_sum.append(0)

        # ---- chunks 1..NCH-2: uniform kt-groups ----
        for c in range(1, NCH - 1):
            c0, w = coff[c], chunks[c]
            for kt in range(KT):
                mm = mmp.tile([P, CW], f32, tag="mm", name=f"mm{kt}_{c}")
                dr_pair(mm, kt, c0, w, first=True)
                bf_sweep(mm, kt, c0, w)
                if kt == 2 and pend_sum:
                    cc = pend_sum.pop(0)
                    pend_ship.append((cc, psum_sum(cc)))
                if kt == 4 and pend_ship:
                    cc, sc = pend_ship.pop(0)
                    ship(cc, sc)
                en = enp.tile([P, CW], bf16, tag="en", name=f"en{kt}_{c}")
                tanh_fold(mm, en, kt, c)
            pend_sum.append(c)

        # ---- tail chunk: Va-fold on PE (no serial DVE chain). The fold
        # matmuls (va.T @ en_kt accumulated into one PSUM sliver) ride the
        # tail matmul stream with a 2-group lag so only kt6/kt7's folds can
        # ever wait on a tanh at the very end. ----
        ct = NCH - 1
        c0, w = coff[ct], chunks[ct]
        en_t = {}
        sct = scp.tile([1, CW], f32, tag="sc")

        def fold_mm(kt):
            nc.tensor.matmul(sct[:, 0:w], vab_t[:, kt : kt + 1], en_t[kt][:, 0:w],
                             start=(kt == 0), stop=(kt == KT - 1))

        for kt in range(KT):
            mm = mmp.tile([P, CW], f32, tag="mm", name=f"mm{kt}_{ct}")
            dr_pair(mm, kt, c0, w, first=True)
            bf_sweep(mm, kt, c0, w)
            if kt == 2 and pend_sum:
                cc = pend_sum.pop(0)
                pend_ship.append((cc, psum_sum(cc)))
            if kt == 4 and pend_ship:
                cc, sc = pend_ship.pop(0)
                ship(cc, sc)
            # dedicated tiles: an enp-pool tile here would chain the tail
            # tanhs behind the fold-matmuls' consumption of earlier slots
            en = const.tile([P, CW], bf16, tag=f"ent{kt}", name=f"en{kt}_t")
            en_t[kt] = en
            for (r0, r1, sl) in act_ranges(ct):
                nc.scalar.activation(en[:, r0 - c0 : r1 - c0],
                                     mm[:, r0 - c0 : r1 - c0], Tanh,
                                     bias=cbias_col(kt, sl), scale=1.0 / WSCALE)
            if kt >= 2:
                fold_mm(kt - 2)
        # drain any leftover mid-chunk bookkeeping
        for cc, sc in pend_ship:
            ship(cc, sc)
        pend_ship = []
        for cc in pend_sum:
            if cc != ct:
                sc = psum_sum(cc)
                ship(cc, sc)
        pend_sum = []
        fold_mm(KT - 2)
        fold_mm(KT - 1)
        nc.vector.tensor_copy(prow[0:1, c0 : c0 + w], sct[:, 0:w])
        A.dma_start(out[0:1, shipped[0] : T], prow[0:1, shipped[0] : T])

    return nc


def make_nc(T, bounds):
    from concourse import bacc

    nc = bacc.Bacc("TRN2", target_bir_lowering=False)
    build_kernel(nc, T, bounds)
    nc.compile()
    return nc


def host_prep(decoder_hidden, encoder_outputs, mask, Wa_w, Wa_b, Ua_w, Ua_b, Va_w,
              n_cores=NCORES):
    """Compact, slot-balance, quantize and lay out inputs for the device.

    Returns (in_maps, T, bounds, placement) where placement[core] is a list
    of (batch, n_kept, col_offset) per slot for the host-side scatter."""
    bf = ml_dtypes.bfloat16
    e4 = ml_dtypes.float8_e4m3fn
    b_total, s, h = encoder_outputs.shape

    mask_np = np.asarray(mask)
    idxs = [np.nonzero(mask_np[b])[0] for b in range(b_total)]
    s_eff = np.array([len(i) for i in idxs])

    # sort batches by length desc; slot j takes ranks [8j, 8j+8), one per core
    order = np.argsort(-s_eff, kind="stable")
    L = []
    assign = []  # assign[j][c] = batch id
    for j in range(NSLOT):
        grp = order[n_cores * j : n_cores * (j + 1)]
        L.append(int(min(-(-max(s_eff[grp].max(), 1) // 16) * 16, s)))
        assign.append(list(grp))
    T = sum(L)
    bounds = [0]
    for lj in L:
        bounds.append(bounds[-1] + lj)

    f32 = np.float32
    dec = np.asarray(decoder_hidden, f32)
    enc = np.asarray(encoder_outputs, f32)
    Ua = np.asarray(Ua_w, f32)
    cb_full = (dec @ np.asarray(Wa_w, f32) + np.asarray(Wa_b, f32)
               + np.asarray(Ua_b, f32))  # [B, H]

    # weights, replicated
    ua_s = WSCALE * Ua
    ua8 = np.ascontiguousarray(
        ua_s[0 : NF8 * P, :].reshape(NF8, P, KT, P).transpose(1, 2, 0, 3)
    ).astype(e4)  # [P, KT, NF8, P]
    uab = np.ascontiguousarray(
        ua_s[NF8 * P :, :].reshape(NB16, P, KT, P).transpose(1, 2, 0, 3)
        .reshape(P, KT, NB16 * P)
    ).astype(bf)
    va_sb = np.ascontiguousarray(np.asarray(Va_w, f32).reshape(KT, P).T)  # [P, KT]
    vab = va_sb.astype(bf)

    in_maps = []
    placement = []
    for c in range(n_cores):
        enc8 = np.zeros((P, NF8, T), e4)
        encb = np.zeros((P, NB16, T), bf)
        cbias = np.zeros((P, KT * NSLOT), f32)
        place = []
        for j in range(NSLOT):
            b = assign[j][c]
            n = min(int(s_eff[b]), L[j])
            o = bounds[j]
            et = enc[b][idxs[b][:n]].T  # [H, n]
            enc8[:, :, o : o + n] = et[0 : NF8 * P].reshape(NF8, P, n).transpose(1, 0, 2).astype(e4)
            encb[:, :, o : o + n] = et[NF8 * P :].reshape(NB16, P, n).transpose(1, 0, 2).astype(bf)
            cbias[:, j::NSLOT] = cb_full[b].reshape(KT, P).T  # col kt*NSLOT+j
            place.append((int(b), n, o))
        cbf = np.concatenate([cbias, va_sb], axis=1)
        in_maps.append(dict(enc8=enc8, encb=encb, ua8=ua8, uab=uab,
                            cbf=cbf, vab=vab))
        placement.append(place)
    return in_maps, T, bounds, placement


def scatter_output(core_outs, placement, idxs_all, b_total, s_full):
    """Softmax the per-core score rows (host, float64) and scatter back to
    the full [B, S] output. Masked positions are exactly 0.0, matching the
    reference's underflowed exp."""
    out = np.zeros((b_total, s_full), np.float32)
    for c, row in enumerate(core_outs):
        for (b, n, o) in placement[c]:
            if n == 0:
                continue
            r = row[o : o + n].astype(np.float64)
            e = np.exp(r - r.max())
            out[b, idxs_all[b][:n]] = (e / e.sum()).astype(np.float32)
    return out


_NC_CACHE = {}


def run(inputs, trace=False, **spmd_kwargs):
    """Run on the 8 NeuronCores; returns (full_output, BassKernelResults)."""
    from concourse.bass_utils import run_bass_kernel_spmd

    mask_np = np.asarray(inputs["mask"])
    idxs_all = [np.nonzero(mask_np[b])[0] for b in range(mask_np.shape[0])]
    in_maps, T, bounds, placement = host_prep(
        inputs["decoder_hidden"],
        inputs["encoder_outputs"],
        inputs["mask"],
        inputs["Wa_w"],
        inputs["Wa_b"],
        inputs["Ua_w"],
        inputs["Ua_b"],
        inputs["Va_w"],
    )
    key = (T, tuple(bounds))
    if key not in _NC_CACHE:
        _NC_CACHE[key] = make_nc(T, bounds)
    nc = _NC_CACHE[key]
    res = run_bass_kernel_spmd(
        nc, in_maps, list(range(NCORES)), trace=trace, **spmd_kwargs
    )
    outs = [np.asarray(r["scores"], np.float32).reshape(-1) for r in res.results]
    return scatter_output(outs, placement, idxs_all, B, S), res


def kernel(**inputs) -> np.ndarray:
    out, _ = run(inputs, trace=False)
    return out
